# revision 1
# baseline (speedup 1.0000x reference)
"""Trainium2 Bass kernel for nn_CausalSelfAttention_42039139893449.

Differential causal self-attention block:
  qkv = x @ ternary(W_qkv).T ; qk rmsnorm ; rope ; q*gain ; GQA expand
  y1/y2 = causal attention over head halves ; y = [y1-lam*y2, y1+lam*y2]
  out = rmsnorm(y) @ ternary(W_proj).T

Sharding over 8 NeuronCores: batch (4) x head-halves (2).
Per core: QKV projection for its 8 q-heads / 2 kv-heads, differential
causal attention, pairwise AllGather of attention outputs within the
batch pair, output projection for half of the output columns (final
RMSNorm is folded into the projection epilogue as a per-token scale).

Host-side prep (ternary weight quantization, transposes, fp16 hi/lo
splits, rope tables, causal mask) is input preprocessing; all module
FLOPs run on device.

Precision strategy: Q/K projection and scores run as fp16 hi/lo 3-pass
matmuls (fp32-quality at 3 cycles/row); the V projection, PV matmul and
output projection run as float32r (1 cycle/row).

Layouts: activations stay "transposed" on device -- [head-dim on
partitions, tokens on free dim] -- so no on-device transposes are needed:
  scores^T[key, q] = k^T.T @ q^T   (contraction over head-dim halves)
  y^T[d, q]        = [v|1].T @ p^T (contraction over keys; row 64 of the
                                    output is the softmax denominator)
  proj uses y^T tiles directly as lhsT.
Head-dim halves are packed into partitions 0-63 / 64-127 of shared
tiles; the two halves' 64-contraction score matmuls occupy disjoint PE
row groups and run concurrently. Rope uses a partition-swapped copy and
a sign-folded sin table.
"""
import sys

if "/opt/trn_rl_repo" not in sys.path:
    sys.path.insert(0, "/opt/trn_rl_repo")

import numpy as np

import concourse.bass as bass
import concourse.mybir as mybir
import concourse.tile as tile
from concourse import bacc
from concourse import bass_utils

# ---- problem constants (hardcoded) ----
B, S, DIM = 4, 1024, 2048
H, KVH, HD = 16, 4, 128
HALF = HD // 2          # 64
GS = 64
ROPE_BASE = 10000.0
QS, KVS = H * HD, KVH * HD   # 2048, 512
N_CORES = 8
HL = H // 2              # 8 q heads per core
KVL = KVH // 2           # 2 kv heads per core
REP = H // KVH           # 4
EPS = float(np.finfo(np.float32).eps)
P = 128
KC = DIM // P            # 16 contraction chunks
TT = S // P              # 8 token tiles / key chunks
FTOT = HL + KVL          # 10 q+k feature tiles per core
QKCOLS = FTOT * HD       # 1280 q+k feature cols per core
VCOLS = KVL * HD         # 256
OCOLS = DIM // 2         # 1024 output cols per core

f32 = mybir.dt.float32
f16 = mybir.dt.float16
f32r = mybir.dt.float32r
AF = mybir.ActivationFunctionType

_CACHE = {}


# ---------------- host-side preprocessing ----------------

def _ternary_quant(w):
    wg = w.reshape(-1, GS).astype(np.float32)
    scale = np.clip(np.mean(np.abs(wg), axis=-1, keepdims=True), 1e-8, None)
    scale = scale.astype(np.float32)
    q = np.clip(np.round(wg / scale), -1.0, 1.0).astype(np.float32)
    return (q * scale).reshape(w.shape).astype(np.float32)


def _f16_split(x):
    hi = x.astype(np.float16)
    lo = (x.astype(np.float32) - hi.astype(np.float32)).astype(np.float16)
    return np.ascontiguousarray(hi), np.ascontiguousarray(lo)


def _rope_tables():
    inv_freq = 1.0 / (ROPE_BASE ** (np.arange(0, HD, 2, dtype=np.float32) / HD))
    freqs = np.arange(S, dtype=np.float32)[:, None] * inv_freq[None, :].astype(np.float32)
    cos = np.cos(freqs).astype(np.float32).T   # [64, S]
    sin = np.sin(freqs).astype(np.float32).T
    # packed for the partition-swap rope: [cos; cos], [sin; -sin]
    cpack = np.concatenate([cos, cos], axis=0)
    spack = np.concatenate([sin, -sin], axis=0)
    return np.ascontiguousarray(cpack), np.ascontiguousarray(spack)  # [128, S]


# ---------------- device program ----------------

def _build_program():
    key = ("v2", bool(globals().get("NO_COLLECTIVE", False)))
    if key in _CACHE:
        return _CACHE[key]

    nc = bacc.Bacc("TRN2", target_bir_lowering=False, debug=False,
                   num_devices=N_CORES)

    def din(name, shape, dt_):
        return nc.dram_tensor(name, shape, dt_, kind="ExternalInput").ap()

    xh_d = din("xT_hi", [DIM, S], f16)
    xl_d = din("xT_lo", [DIM, S], f16)
    wh_d = din("wqkT_hi", [DIM, QKCOLS], f16)
    wl_d = din("wqkT_lo", [DIM, QKCOLS], f16)
    xr_d = din("xT_r", [DIM, S], f32r)
    wv_d = din("wvT_r", [DIM, VCOLS], f32r)
    wp_d = din("wpT", [DIM, OCOLS], f32r)
    cos_d = din("cpack", [P, S], f32)
    sin_d = din("spack", [P, S], f32)
    gain_d = din("gain10", [FTOT, 1], f32)
    lam_d = din("lam8", [HL, 1], f32)
    mask_d = din("dmask", [P, P], f32)

    out_d = nc.dram_tensor("out", [S, OCOLS], f32, kind="ExternalOutput").ap()

    def mm3(ps, lhs_pair, rhs_pair, first, last):
        """f16 hi/lo 3-pass matmul accumulating into ps."""
        lh, ll = lhs_pair
        rh, rl = rhs_pair
        nc.tensor.matmul(ps, lh, rh, start=first, stop=False,
                         skip_group_check=True)
        nc.tensor.matmul(ps, lh, rl, start=False, stop=False,
                         skip_group_check=True)
        nc.tensor.matmul(ps, ll, rh, start=False, stop=last,
                         skip_group_check=True)

    with tile.TileContext(nc) as tc:
        with (
            tc.tile_pool(name="const", bufs=1) as cp,
            tc.tile_pool(name="dram", bufs=1, space="DRAM") as dp,
        ):
            # ---- small constants ----
            dmask = cp.tile([P, P], f32)
            nc.sync.dma_start(dmask[:], mask_d[:])
            lam8 = cp.tile([HL, 1], f32)
            nc.sync.dma_start(lam8[:], lam_d[:])
            gain10 = cp.tile([FTOT, 1], f32)
            nc.sync.dma_start(gain10[:], gain_d[:])
            ones128 = cp.tile([P, 1], f32)
            nc.vector.memset(ones128[:], 1.0)
            epsc = cp.tile([P, 1], f32)
            nc.vector.memset(epsc[:], EPS)
            sgn = cp.tile([P, 1], f32)
            nc.vector.memset(sgn[0:HALF, :], -1.0)
            nc.vector.memset(sgn[HALF:P, :], 1.0)

            ssq_dram = dp.tile([FTOT, S], f32)
            rr_dram = dp.tile([FTOT, S], f32)
            den_dram = dp.tile([2 * HL, S], f32)
            agin = dp.tile([HL * HD, S], f32r)
            agout = dp.tile([H * HD, S], f32r)

            yraw_dram = dp.tile([P, HL, S], f32)

            # ---- long-lived tiles, allocated in LIFO (stack) order ----
            den16, free_den16 = tc.tile([32 + HL, S], f32, name="den16")
            qk16h, free_qk16h = tc.tile([P, FTOT, S], f16, name="qk16h")
            qk16l, free_qk16l = tc.tile([P, FTOT, S], f16, name="qk16l")
            vplus, free_vplus = tc.tile([P, KVL, 2, TT, HALF + 1], f32r, name="vplus")
            nc.vector.tensor_copy(
                vplus[:, :, :, :, HALF:HALF + 1],
                ones128.rearrange("p (a b c o) -> p a b c o", a=1, b=1, c=1)
                .to_broadcast([P, KVL, 2, TT, 1]))
            qkT, free_qkT = tc.tile([P, FTOT, S], f32, name="qkT")
            cpk, free_cpk = tc.tile([P, S], f32, name="cpk")
            spk, free_spk = tc.tile([P, S], f32, name="spk")
            nc.sync.dma_start(cpk[:], cos_d[:])
            nc.sync.dma_start(spk[:], sin_d[:])

            # ====== stage A: QKV projection ======
            with (
                tc.tile_pool(name="xw", bufs=1) as xw,
                tc.tile_pool(name="psA", bufs=1, space="PSUM") as psA,
            ):
                for th in range(2):
                    t0 = th * 512
                    xh = xw.tile([P, KC, 512], f16, tag="xh", bufs=1)
                    xl = xw.tile([P, KC, 512], f16, tag="xl", bufs=1)
                    nc.sync.dma_start(
                        xh[:], xh_d[:, t0:t0 + 512].rearrange("(c p) t -> p c t", p=P))
                    nc.sync.dma_start(
                        xl[:], xl_d[:, t0:t0 + 512].rearrange("(c p) t -> p c t", p=P))
                    for ft in range(FTOT):
                        c0 = ft * P
                        wth = xw.tile([P, KC, P], f16, tag="wth", bufs=2)
                        wtl = xw.tile([P, KC, P], f16, tag="wtl", bufs=2)
                        nc.sync.dma_start(
                            wth[:], wh_d[:, c0:c0 + P].rearrange("(c p) f -> p c f", p=P))
                        nc.sync.dma_start(
                            wtl[:], wl_d[:, c0:c0 + P].rearrange("(c p) f -> p c f", p=P))
                        ps = psA.tile([P, 512], f32, tag="mm", bufs=4)
                        for c in range(KC):
                            mm3(ps[:], (wth[:, c], wtl[:, c]),
                                (xh[:, c], xl[:, c]),
                                c == 0, c == KC - 1)
                        nc.vector.tensor_copy(qkT[:, ft, t0:t0 + 512], ps[:])
                        # rms stats: sum of squares over head-dim (partitions)
                        sq = xw.tile([P, 512], f32, tag="sq", bufs=1)
                        nc.scalar.activation(sq[:], ps[:], AF.Square)
                        pss = psA.tile([P, 512], f32, tag="ssq", bufs=2)
                        nc.tensor.matmul(pss[0:1, :], ones128[:], sq[:],
                                         start=True, stop=True,
                                         skip_group_check=True)
                        stg = xw.tile([1, 512], f32, tag="stg", bufs=2)
                        nc.vector.tensor_copy(stg[:], pss[0:1, :])
                        nc.sync.dma_start(ssq_dram[ft:ft + 1, t0:t0 + 512], stg[:])

                # V projection in f32r -> [tokens, feats] into vplus
                wvr = xw.tile([P, KC, VCOLS], f32r)
                nc.sync.dma_start(wvr[:], wv_d.rearrange("(c p) f -> p c f", p=P))
                for t_ in range(TT):
                    xr = xw.tile([P, KC, P], f32r, tag="xr", bufs=2)
                    nc.sync.dma_start(
                        xr[:], xr_d[:, t_ * P:(t_ + 1) * P].rearrange("(c p) t -> p c t", p=P))
                    psv = psA.tile([P, VCOLS], f32, tag="mmv", bufs=2)
                    for c in range(KC):
                        nc.tensor.matmul(psv[:], xr[:, c], wvr[:, c],
                                         start=(c == 0), stop=(c == KC - 1),
                                         skip_group_check=True)
                    for kv in range(KVL):
                        for hf in range(2):
                            nc.vector.tensor_copy(
                                vplus[:, kv, hf, t_, 0:HALF],
                                psv[:, kv * HD + hf * HALF: kv * HD + (hf + 1) * HALF])

            # ====== stage B: rr + rope + scale + f16 split ======
            ssq10, free_ssq10 = tc.tile([FTOT, S], f32, name="ssq10")
            nc.sync.dma_start(ssq10[:], ssq_dram[:])
            nc.scalar.activation(ssq10[:], ssq10[:], AF.Sqrt, scale=1.0 / HD,
                                 bias=epsc[0:FTOT, 0:1])
            nc.vector.reciprocal(ssq10[:], ssq10[:])
            nc.vector.tensor_scalar_mul(ssq10[:], ssq10[:], gain10[:, 0:1])
            nc.sync.dma_start(rr_dram[:], ssq10[:])
            free_ssq10()


            with tc.tile_pool(name="ropep", bufs=1) as ropep:
                for ft in range(FTOT):
                    qks = ropep.tile([P, S], f32, tag="qks", bufs=2)
                    nc.sync.dma_start(qks[0:HALF, :], qkT[HALF:P, ft, :])
                    nc.sync.dma_start(qks[HALF:P, :], qkT[0:HALF, ft, :])
                    rrb = ropep.tile([P, S], f32, tag="rrb", bufs=2)
                    nc.sync.dma_start(rrb[:],
                                      rr_dram[ft:ft + 1, :].to_broadcast([P, S]))
                    # rope: qkT = qkT*cpack + swap(qkT)*spack, then *rr
                    nc.vector.tensor_mul(qks[:], qks[:], spk[:])
                    nc.vector.tensor_mul(qkT[:, ft, :], qkT[:, ft, :], cpk[:])
                    nc.vector.tensor_add(qkT[:, ft, :], qkT[:, ft, :], qks[:])
                    nc.vector.tensor_mul(qkT[:, ft, :], qkT[:, ft, :], rrb[:])
                    nc.vector.tensor_copy(qk16h[:, ft, :], qkT[:, ft, :])
                    nc.vector.tensor_sub(qk16l[:, ft, :], qkT[:, ft, :],
                                         qk16h[:, ft, :])
            free_spk()
            free_cpk()
            free_qkT()

            # ====== stage C: differential causal attention ======
            # halves packed: half s_ of head h lives at partitions s_*64..
            with (
                tc.tile_pool(name="psC", bufs=1, space="PSUM") as psC,
                tc.tile_pool(name="awp", bufs=1) as awp,
            ):
                for h in range(HL):
                    kv = h // REP
                    yps = [psC.tile([HALF + 1, 512], f32, tag=f"y{i}",
                                    bufs=1, name=f"yps{i}")
                           for i in range(4)]  # index: half*2 + seg
                    seg_open = [False] * 4
                    for kc in range(TT):
                        k0 = kc * P
                        segs = []
                        if k0 < 512:
                            segs.append((0, k0, 512 - k0))
                        segs.append((1, max(512, k0), 1024 - max(512, k0)))
                        for (si, q0, w) in segs:
                            sts = []
                            # the two halves' score matmuls occupy disjoint PE
                            # row groups (0-63 / 64-127) -> run concurrently
                            for s_ in range(2):
                                pb = s_ * HALF
                                st = psC.tile([P, 512], f32, tag="sc", bufs=4,
                                              name=f"st{s_}")
                                lp = (qk16h[pb:pb + HALF, HL + kv, k0:k0 + P],
                                      qk16l[pb:pb + HALF, HL + kv, k0:k0 + P])
                                rp_ = (qk16h[pb:pb + HALF, h, q0:q0 + w],
                                       qk16l[pb:pb + HALF, h, q0:q0 + w])
                                mm3(st[:, 0:w], lp, rp_, True, True)
                                sts.append(st)
                            for s_ in range(2):
                                st = sts[s_]
                                gi = s_ * 2 + si
                                pt = awp.tile([P, 512], f32r, tag="pt", bufs=4)
                                nc.scalar.activation(pt[:, 0:w], st[:, 0:w], AF.Exp,
                                                     scale=float(1.0 / np.sqrt(HALF)))
                                if q0 == k0:
                                    nc.vector.tensor_mul(pt[:, 0:P], pt[:, 0:P],
                                                         dmask[:])
                                nc.tensor.matmul(
                                    yps[gi][:, q0 - si * 512: q0 - si * 512 + w],
                                    vplus[:, kv, s_, kc, :], pt[:, 0:w],
                                    start=not seg_open[gi],
                                    stop=(kc == TT - 1 if si == 1 else kc == 3),
                                    skip_group_check=True)
                                seg_open[gi] = True
                    for s_ in range(2):
                        pb = s_ * HALF
                        dtmp = awp.tile([HALF + 1, S], f32, tag=f"dtmp{s_}",
                                        bufs=2, name=f"dtmp{s_}")
                        for si in range(2):
                            gi = s_ * 2 + si
                            sl = slice(si * 512, (si + 1) * 512)
                            ystg = awp.tile([HALF, 512], f32, tag="ystg", bufs=3)
                            nc.vector.tensor_copy(ystg[:], yps[gi][0:HALF, :])
                            nc.sync.dma_start(yraw_dram[pb:pb + HALF, h, sl],
                                              ystg[:])
                            nc.vector.tensor_copy(dtmp[HALF:HALF + 1, sl],
                                                  yps[gi][HALF:HALF + 1, :])
                        drow = s_ * 32 + h
                        nc.sync.dma_start(den16[drow:drow + 1, :],
                                          dtmp[HALF:HALF + 1, :])
            free_vplus()
            free_qk16l()
            free_qk16h()

            # reciprocal of denominators; fold lambda into half-2 rows
            nc.vector.reciprocal(den16[0:HL, :], den16[0:HL, :])
            nc.vector.reciprocal(den16[32:32 + HL, :], den16[32:32 + HL, :])
            nc.vector.tensor_scalar_mul(den16[32:32 + HL, :],
                                        den16[32:32 + HL, :], lam8[:, 0:1])
            nc.sync.dma_start(den_dram[0:HL, :], den16[0:HL, :])
            nc.sync.dma_start(den_dram[HL:2 * HL, :], den16[32:32 + HL, :])
            free_den16()

            # ====== combine: yA = y1*r1 - lam*y2*r2 ; yB = y1*r1 + lam*y2*r2
            # (wpT prefetch starts here so the weights arrive during the
            #  collective)
            wo_ctx = tc.tile_pool(name="wo_pool", bufs=1)
            wo = wo_ctx.__enter__()
            wpTs = []
            for ns in range(2):
                wpT = wo.tile([P, KC, 512], f32r, tag=f"wpT{ns}", bufs=1,
                              name=f"wpT{ns}")
                nc.sync.dma_start(
                    wpT[:], wp_d[:, ns * 512:(ns + 1) * 512].rearrange("(c p) f -> p c f", p=P))
                wpTs.append(wpT)
            yout, free_yout = tc.tile([P, HL, S], f32r, name="yout")
            yswap, free_yswap = tc.tile([P, HL, S], f32, name="yswap")
            yr2, free_yr2 = tc.tile([P, HL, S], f32, name="yr2")
            rb, free_rb = tc.tile([P, HL, S], f32, name="rb")
            for h in range(HL):
                nc.sync.dma_start(yr2[:, h, :], yraw_dram[:, h, :])
                nc.sync.dma_start(rb[0:HALF, h, :],
                                  den_dram[h:h + 1, :].to_broadcast([HALF, S]))
                nc.sync.dma_start(rb[HALF:P, h, :],
                                  den_dram[HL + h:HL + h + 1, :].to_broadcast([HALF, S]))
            nc.vector.tensor_mul(yr2[:], yr2[:], rb[:])
            free_rb()
            nc.sync.dma_start(yswap[0:HALF, :, :], yr2[HALF:P, :, :])
            nc.sync.dma_start(yswap[HALF:P, :, :], yr2[0:HALF, :, :])
            nc.vector.tensor_scalar_mul(yswap[:], yswap[:], sgn[:, 0:1])
            nc.vector.tensor_add(yout[:], yswap[:], yr2[:])
            free_yr2()
            free_yswap()
            nc.sync.dma_start(agin.rearrange("(h d) t -> d h t", d=HD), yout[:])

            # local final-rms stats from yout; pair-sum via tiny AllReduce
            ssqy_in = dp.tile([P, TT], f32)
            ssqy_out = dp.tile([P, TT], f32)
            with (
                tc.tile_pool(name="psS", bufs=1, space="PSUM") as psS,
                tc.tile_pool(name="sql_pool", bufs=2) as sql,
            ):
                # separate psum tiles per token tile: a shared bank would lose
                # accumulation state on each start=True whole-bank bit-clear
                psqs = [psS.tile([P, 1], f32, tag=f"psq{t_}", bufs=1,
                                 name=f"psq{t_}")
                        for t_ in range(TT)]
                for c in range(HL):
                    sqy = sql.tile([P, S], f32, tag="sqy")
                    nc.scalar.activation(sqy[:], yout[:, c, :].bitcast(f32),
                                         AF.Square)
                    for t_ in range(TT):
                        nc.tensor.matmul(psqs[t_][:],
                                         sqy[:, t_ * P:(t_ + 1) * P],
                                         ones128[:], start=(c == 0),
                                         stop=(c == HL - 1),
                                         skip_group_check=True)
                ssql = sql.tile([P, TT], f32)
                for t_ in range(TT):
                    nc.vector.tensor_copy(ssql[:, t_:t_ + 1], psqs[t_][:])
                nc.sync.dma_start(ssqy_in[:], ssql[:])
            free_yout()

            groups = [[2 * i, 2 * i + 1] for i in range(N_CORES // 2)]
            if globals().get("NO_COLLECTIVE", False):
                # timing-analysis stubs: TimelineSim can't simulate collectives
                nc.sync.dma_start(ssqy_out[:], ssqy_in[:])
                nc.sync.dma_start(agout[0:HL * HD, :], agin[:])
                nc.sync.dma_start(agout[HL * HD:, :], agin[:])
            else:
                nc.gpsimd.collective_compute(
                    "AllReduce", mybir.AluOpType.add,
                    ins=[ssqy_in.opt()], outs=[ssqy_out.opt()],
                    replica_groups=groups,
                )
                nc.gpsimd.collective_compute(
                    "AllGather", mybir.AluOpType.bypass,
                    ins=[agin.opt()], outs=[agout.opt()],
                    replica_groups=groups,
                )

            # ====== stage D: projection (rmsnorm folded via rry) ======
            yfull, free_yfull = tc.tile([P, H, S], f32r, name="yfull")
            for cc in range(4):
                nc.sync.dma_start(
                    yfull[:, cc * 4:(cc + 1) * 4, :],
                    agout[cc * 4 * HD:(cc + 1) * 4 * HD, :].rearrange(
                        "(h d) t -> d h t", d=HD))

            rry, free_rry = tc.tile([P, TT], f32, name="rry")
            nc.sync.dma_start(rry[:], ssqy_out[:])
            nc.scalar.activation(rry[:], rry[:], AF.Sqrt, scale=1.0 / DIM,
                                 bias=epsc[:, 0:1])
            nc.vector.reciprocal(rry[:], rry[:])

            with tc.tile_pool(name="psD2", bufs=1, space="PSUM") as psD2:
                for ns in range(2):
                    wpT = wpTs[ns]
                    for tb in range(2):
                        psos = [psD2.tile([P, 512], f32, tag=f"pj{i}", bufs=2,
                                          name=f"pso{i}")
                                for i in range(4)]
                        for c in range(KC):
                            for i in range(4):
                                t_ = tb * 4 + i
                                nc.tensor.matmul(
                                    psos[i][:], yfull[:, c, t_ * P:(t_ + 1) * P],
                                    wpT[:, c, :], start=(c == 0),
                                    stop=(c == KC - 1), skip_group_check=True)
                        for i in range(4):
                            t_ = tb * 4 + i
                            osb = wo.tile([P, 512], f32, tag="osb", bufs=3)
                            nc.vector.tensor_scalar_mul(osb[:], psos[i][:],
                                                        rry[:, t_:t_ + 1])
                            nc.sync.dma_start(
                                out_d[t_ * P:(t_ + 1) * P, ns * 512:(ns + 1) * 512],
                                osb[:])
            free_rry()
            free_yfull()
            wo_ctx.__exit__(None, None, None)

    nc.compile()
    _CACHE[key] = nc
    return nc


# ---------------- host wrapper ----------------

def _prep_inputs(x, w_qkv, w_proj, q_gain, diff_lambda):
    x = np.asarray(x, dtype=np.float32)
    wq = _ternary_quant(np.asarray(w_qkv, dtype=np.float32))
    wp = _ternary_quant(np.asarray(w_proj, dtype=np.float32))
    q_gain = np.asarray(q_gain, dtype=np.float32)
    diff_lambda = np.asarray(diff_lambda, dtype=np.float32)
    cpack, spack = _rope_tables()

    # causal mask for diagonal 128x128 blocks in scores^T layout:
    # element (key p, query j) valid iff j >= p
    dmask = (np.arange(P)[None, :] >= np.arange(P)[:, None]).astype(np.float32)
    dmask = np.ascontiguousarray(dmask)

    in_maps = []
    for core in range(N_CORES):
        b, hh = core // 2, core % 2
        q_rows = wq[hh * HL * HD:(hh + 1) * HL * HD]                   # [1024, 2048]
        k_rows = wq[QS + hh * KVL * HD: QS + (hh + 1) * KVL * HD]      # [256, 2048]
        v_rows = wq[QS + KVS + hh * KVL * HD: QS + KVS + (hh + 1) * KVL * HD]
        wqk_T = np.ascontiguousarray(np.concatenate([q_rows, k_rows], axis=0).T)
        wv_T = np.ascontiguousarray(v_rows.T)                          # [2048, 256]
        xT = np.ascontiguousarray(x[b].T)                              # [2048, 1024]
        wpT = np.ascontiguousarray(wp[hh * OCOLS:(hh + 1) * OCOLS].T)  # [2048, 1024]

        gain10 = np.concatenate([q_gain[hh * HL:(hh + 1) * HL],
                                 np.ones(KVL, np.float32)]).reshape(FTOT, 1)
        lam8 = diff_lambda[hh * HL:(hh + 1) * HL].reshape(HL, 1).astype(np.float32)

        xh, xl = _f16_split(xT)
        wh, wl = _f16_split(wqk_T)
        m = {
            "xT_hi": xh, "xT_lo": xl,
            "wqkT_hi": wh, "wqkT_lo": wl,
            "xT_r": xT, "wvT_r": wv_T,
            "wpT": wpT,
            "cpack": cpack, "spack": spack,
            "gain10": np.ascontiguousarray(gain10.astype(np.float32)),
            "lam8": np.ascontiguousarray(lam8),
            "dmask": dmask,
        }
        in_maps.append(m)
    return in_maps


def kernel(x, w_qkv, w_proj, q_gain, diff_lambda):
    nc = _build_program()
    in_maps = _prep_inputs(x, w_qkv, w_proj, q_gain, diff_lambda)
    last_err = None
    for attempt in range(3):
        try:
            res = bass_utils.run_bass_kernel_spmd(
                nc, in_maps, core_ids=list(range(N_CORES)))
            break
        except Exception as e:  # transient device wedges recover on retry
            last_err = e
            import time as _time
            _time.sleep(2.0)
    else:
        raise last_err
    out = np.empty((B, S, DIM), dtype=np.float32)
    for core in range(N_CORES):
        b, hh = core // 2, core % 2
        out[b, :, hh * OCOLS:(hh + 1) * OCOLS] = res.results[core]["out"]
    return out

